# revision 2
# baseline (speedup 1.0000x reference)
"""YOLOv1 loss kernel for 8 Trainium2 NeuronCores.

Strategy (data parallel over batch):
  - Shard pred/target [16384,7,7,30] along batch across 8 cores (2048 each).
  - Per core: 2048*49 = 100352 cells, each a 30-float record.
    Split channels: "box" block ch0:10 kept f32, "class" block ch10:30
    downcast to bf16 on host (class loss is argmax-free, tolerates bf16;
    cuts DMA bytes from 120 to 80 per cell).
  - Layout: cells mapped to [T=4 tiles][P=128 partitions][F=196 cells].
    All per-cell math is [P, F]-shaped elementwise work spread across
    DVE / ACT / Pool engines; per-(partition,tile) partial sums of the
    9 loss sub-terms are emitted via accum_out into a [P, T*10] tile,
    DMA'd out once. Host does the final tiny reduction in float64.

IoU via interval-overlap identity:
  overlap_x = relu(0.5*(wp+wt) - |cxp - cxt|)  (centers pre-scaled by 1/14)
Responsible-box select: sel = (iou1 > iou0) as 0/1 mask; every
sel-dependent loss term is accumulated separately for box0 (mask
coo*(1-sel)) and box1 (mask coo*sel), so no blend ops are needed.
"""

import sys

if "/opt/trn_rl_repo" not in sys.path:
    sys.path.insert(0, "/opt/trn_rl_repo")

import numpy as np
import ml_dtypes

import concourse.bass as bass
import concourse.tile as tile
from concourse import mybir
from concourse.bass_utils import run_bass_kernel_spmd

BF16 = ml_dtypes.bfloat16

NCORES = 8
B, S, C = 16384, 7, 30
BS = B // NCORES            # 2048 batches per core
NCELL = BS * S * S          # 100352 cells per core
P = 128
FPP = NCELL // P            # 784 cells per partition
INV = 1.0 / 14.0
NACC = 10                   # accum columns per tile (9 used)

f32 = mybir.dt.float32
bf16 = mybir.dt.bfloat16
Alu = mybir.AluOpType
Act = mybir.ActivationFunctionType
Ax = mybir.AxisListType


def build_nc(ncell=NCELL, F=196, fix_waits=True, repeat=0, level=4):
    F2 = 2 * F
    """Build the SPMD Bass program for one core's shard of `ncell` cells."""
    T = ncell // (P * F)
    assert T * P * F == ncell

    nc = bass.Bass(target_bir_lowering=False)
    # merged streams: box = pred ch0:10 | target ch0:10 (f32);
    # cls = pred ch10:30 | target ch10:30 (bf16)
    box = nc.declare_dram_parameter("box", [T * P, F, 20], bf16, isOutput=False)
    clsin = nc.declare_dram_parameter("cls", [T * P, F, 40], bf16, isOutput=False)
    out = nc.declare_dram_parameter("out", [P, T * NACC], f32, isOutput=True)

    ENG_SUB = nc.vector
    ENG_ADD = nc.vector
    ENG_STT = nc.vector
    with tile.TileContext(nc) as tc:
        with tc.tile_pool(name="io", bufs=2) as io, \
             tc.tile_pool(name="tmp", bufs=1) as tmp, \
             tc.tile_pool(name="accp", bufs=1) as accp:
            acc = accp.tile([P, T * NACC], f32)
            nc.vector.memset(acc, 0.0)

            import contextlib
            assert T % 2 == 0
            rep_ctx = tc.For_i(0, repeat, 1) if repeat else contextlib.nullcontext()
            with rep_ctx:
                for it in range(T // 2):
                    r0, r1, r2 = 2 * it * P, (2 * it + 1) * P, (2 * it + 2) * P
                    F = F2
                    mb = io.tile([P, F, 20], bf16, tag="mb")
                    mc = io.tile([P, F, 40], bf16, tag="mc")
                    if level != 5:
                        nc.sync.dma_start(out=mb[:, 0:F // 2, :], in_=box[r0:r1])
                        nc.sync.dma_start(out=mb[:, F // 2:F, :], in_=box[r1:r2])
                        nc.sync.dma_start(out=mc[:, 0:F // 2, :], in_=clsin[r0:r1])
                        nc.sync.dma_start(out=mc[:, F // 2:F, :], in_=clsin[r1:r2])
                    pbt = mb[:, :, 0:10]
                    tbt = mb[:, :, 10:20]
                    scrV = tmp.tile([P, F], bf16, tag="scrV")
                    scrA = scrV
                    scrP = scrV
                    b = it * NACC

                    def vacc(mask, val, col):
                        nc.vector.scalar_tensor_tensor(
                            scrV, mask, 0.0, val, op0=Alu.bypass, op1=Alu.mult,
                            accum_out=acc[:, col:col + 1])

                    coo = tbt[:, :, 4]

                    if level == 0:
                        continue
                    # ---- class block: q = (p-t)^2, squared into a fully
                    # contiguous bf16 tile so the reduce can use packed mode
                    qc = mc[:, :, 20:40]
                    ENG_SUB.tensor_sub(qc, mc[:, :, 0:20], qc)
                    nc.scalar.square(qc, qc)
                    cls = tmp.tile([P, F], f32, tag="cls")
                    nc.vector.tensor_reduce(cls, qc, axis=Ax.X, op=Alu.add)

                    if level >= 1:
                        vacc(mb[:, :, 14], cls, b + 2)
                    if level == 1:
                        continue
                    # ---- box diffs packed: dxy=(d0,d1,d5,d6), dd49=(d4,d9)
                    dxy = tmp.tile([P, F, 4], bf16, tag="dxy")
                    nc.vector.tensor_sub(dxy[:, :, 0:2], pbt[:, :, 0:2], tbt[:, :, 0:2])
                    nc.vector.tensor_sub(dxy[:, :, 2:4], pbt[:, :, 5:7], tbt[:, :, 5:7])
                    dd49 = tmp.tile([P, F, 2], bf16, tag="dd49")
                    nc.vector.tensor_sub(dd49, mb[:, :, 4:10:5], mb[:, :, 14:20:5])
                    e1 = tmp.tile([P, F, 2], bf16, tag="e1")
                    nc.vector.tensor_sub(e1, pbt[:, :, 5:7], tbt[:, :, 0:2])

                    # center distances |d|/14 (read raw diffs before in-place squares)
                    av = tmp.tile([P, F, 4], bf16, tag="av")
                    nc.scalar.activation(av[:, :, 0:2], dxy[:, :, 0:2], Act.Abs, scale=INV)
                    nc.scalar.activation(av[:, :, 2:4], e1, Act.Abs, scale=INV)

                    nc.scalar.square(dxy, dxy)
                    nc.scalar.square(dd49, dd49)

                    f01 = tmp.tile([P, F, 2], bf16, tag="f01")   # xy sq-dist per box
                    nc.vector.tensor_add(f01, dxy[:, :, 0:4:2], dxy[:, :, 1:4:2])
                    snn = tmp.tile([P, F], bf16, tag="snn")
                    nc.vector.tensor_add(snn, dd49[:, :, 0], dd49[:, :, 1])

                    if level == 2:
                        vacc(mb[:, :, 14], snn, b + 1)
                        vacc(mb[:, :, 14], f01[:, :, 0], b + 3)
                        continue

                    # ---- intersection: iw = relu(min(0.5*(wp+wt) - |dc|, wp, wt))
                    sw = tmp.tile([P, F, 4], bf16, tag="sw")     # (b0x,b0y,b1x,b1y)
                    nc.vector.tensor_add(sw[:, :, 0:2], pbt[:, :, 2:4], tbt[:, :, 2:4])
                    nc.vector.tensor_add(sw[:, :, 2:4], pbt[:, :, 7:9], tbt[:, :, 2:4])
                    nc.vector.scalar_tensor_tensor(sw, sw, 0.5, av,
                                                   op0=Alu.mult, op1=Alu.subtract)
                    nc.vector.tensor_tensor(sw[:, :, 0:2], sw[:, :, 0:2], pbt[:, :, 2:4], op=Alu.min)
                    nc.vector.tensor_tensor(sw[:, :, 2:4], sw[:, :, 2:4], pbt[:, :, 7:9], op=Alu.min)
                    nc.vector.tensor_tensor(sw[:, :, 0:2], sw[:, :, 0:2], tbt[:, :, 2:4], op=Alu.min)
                    nc.vector.tensor_tensor(sw[:, :, 2:4], sw[:, :, 2:4], tbt[:, :, 2:4], op=Alu.min)
                    nc.scalar.activation(sw, sw, Act.Relu)
                    inter = tmp.tile([P, F, 2], bf16, tag="inter")
                    nc.vector.tensor_mul(inter, sw[:, :, 0:4:2], sw[:, :, 1:4:2])

                    at = tmp.tile([P, F], bf16, tag="at")
                    nc.vector.tensor_mul(at, tbt[:, :, 2], tbt[:, :, 3])
                    ap01 = tmp.tile([P, F, 2], bf16, tag="ap01")
                    nc.vector.tensor_mul(ap01, mb[:, :, 2:9:5], mb[:, :, 3:9:5])
                    x01 = tmp.tile([P, F, 2], bf16, tag="x01")   # union
                    nc.vector.tensor_sub(x01[:, :, 0], at, inter[:, :, 0])
                    nc.vector.tensor_sub(x01[:, :, 1], at, inter[:, :, 1])
                    nc.vector.tensor_add(x01, ap01, x01)

                    # log-domain iou: avoids the 8-cyc/elem iterative divide on DVE
                    nc.vector.tensor_scalar_max(inter, inter, 1e-30)
                    lnq = tmp.tile([P, F, 2], bf16, tag="lnq")
                    nc.scalar.activation(lnq, inter, Act.Ln)
                    nc.scalar.activation(x01, x01, Act.Ln)
                    nc.vector.tensor_sub(lnq, lnq, x01)         # log-iou per box
                    sel = tmp.tile([P, F], bf16, tag="sel")
                    nc.vector.tensor_tensor(sel, lnq[:, :, 1], lnq[:, :, 0], op=Alu.is_gt)
                    mlog = tmp.tile([P, F], bf16, tag="mlog")
                    nc.vector.tensor_max(mlog, lnq[:, :, 1], lnq[:, :, 0])
                    miou = tmp.tile([P, F], bf16, tag="miou")
                    nc.scalar.activation(miou, mlog, Act.Exp)

                    # contain: (conf_b - max_iou)^2 per box
                    c01 = tmp.tile([P, F, 2], bf16, tag="c01")
                    nc.vector.tensor_sub(c01[:, :, 0], pbt[:, :, 4], miou)
                    nc.vector.tensor_sub(c01[:, :, 1], pbt[:, :, 9], miou)
                    nc.scalar.square(c01, c01)

                    if level == 3:
                        cs1x = tmp.tile([P, F], bf16, tag="cs1x")
                        nc.vector.tensor_mul(cs1x, mb[:, :, 14], sel)
                        vacc(cs1x, c01[:, :, 0], b + 5)
                        vacc(cs1x, snn, b + 1)
                        continue

                    # not-contain: stored cross-paired (p9^2, p4^2) so one stt works
                    npk = tmp.tile([P, F, 2], bf16, tag="npk")
                    nc.scalar.square(npk[:, :, 0], pbt[:, :, 9])
                    nc.scalar.square(npk[:, :, 1], pbt[:, :, 4])

                    # wh: (sqrt(p)-sqrt(t))^2, both boxes packed
                    # sqrt via exp(0.5*ln x): stays in the natural_log_exp
                    # ACT table set (a direct Sqrt would force a ~2.7us
                    # table-set reload per tile)
                    spq = tmp.tile([P, F, 4], bf16, tag="spq")
                    nc.scalar.activation(spq[:, :, 0:2], pbt[:, :, 2:4], Act.Ln)
                    nc.scalar.activation(spq[:, :, 2:4], pbt[:, :, 7:9], Act.Ln)
                    nc.scalar.activation(spq, spq, Act.Exp, scale=0.5)
                    stq = tmp.tile([P, F, 4], bf16, tag="stq")
                    nc.scalar.activation(stq[:, :, 0:2], tbt[:, :, 2:4], Act.Ln)
                    nc.scalar.activation(stq[:, :, 2:4], tbt[:, :, 7:9], Act.Ln)
                    nc.scalar.activation(stq, stq, Act.Exp, scale=0.5)
                    nc.vector.tensor_sub(spq, spq, stq)
                    nc.scalar.square(spq, spq)
                    w01 = tmp.tile([P, F, 2], bf16, tag="w01")
                    nc.vector.tensor_add(w01, spq[:, :, 0:4:2], spq[:, :, 1:4:2])
                    nc.vector.tensor_add(f01, f01, w01)         # loc per box

                    # masks: cs01 = (coo*(1-sel), coo*sel)
                    cs01 = lnq
                    nc.vector.tensor_mul(cs01[:, :, 1], coo, sel)
                    nc.vector.tensor_sub(cs01[:, :, 0], coo, cs01[:, :, 1])

                    # ---- partial sums (accum_out sums the whole free dim, so one
                    # [P,F,2] stt folds both boxes' masked terms into one column)
                    scr2 = tmp.tile([P, F, 2], bf16, tag="scr2")
                    nc.scalar.activation(scrA, snn, Act.Copy, accum_out=acc[:, b + 0:b + 1])
                    vacc(coo, snn, b + 1)
                    vacc(coo, cls, b + 2)
                    nc.vector.scalar_tensor_tensor(
                        scr2, cs01, 0.0, f01, op0=Alu.bypass, op1=Alu.mult,
                        accum_out=acc[:, b + 3:b + 4])
                    nc.vector.scalar_tensor_tensor(
                        scr2, cs01, 0.0, c01, op0=Alu.bypass, op1=Alu.mult,
                        accum_out=acc[:, b + 5:b + 6])
                    nc.vector.scalar_tensor_tensor(
                        scr2, cs01, 0.0, npk, op0=Alu.bypass, op1=Alu.mult,
                        accum_out=acc[:, b + 7:b + 8])

            nc.gpsimd.dma_start(out=out[:, :], in_=acc[:, :])
    if fix_waits:
        _fix_multi_waits(nc)
    return nc


def _fix_multi_waits(nc):
    """Work around a walrus codegen limit: one sync-wait per TPB instruction.

    Tile sometimes attaches 2-3 sem waits to one engine instruction, which
    this toolchain rejects ("Too many sync wait commands"). Two-step fix:
    1. Drop own-engine waits on DVE/Pool ops (those engines execute their
       streams serially -- DVE drains after every op -- so program order
       already guarantees them).
    2. For any remaining multi-wait compute instruction, move all but the
       last wait onto injected same-engine nops placed immediately before
       it (the engine's sequencer executes them in order, so semantics are
       identical).
    """
    import concourse.mybir as _mybir
    from bass_rust import SyncInfo

    self_pfx = {_mybir.EngineType.DVE: "DVE_", _mybir.EngineType.Pool: "Pool_"}
    blocks = [bb for fn in nc.m.functions for bb in fn.blocks]
    nseq = [0]

    def make_wait(eng, w):
        nseq[0] += 1
        ev = _mybir.InstEventSemaphore(name=f"W-split-{nseq[0]}")
        ev.engine = eng
        ev.sync_info = SyncInfo(on_wait=[w], on_update=[])
        return ev

    for bb in blocks:
        i = 0
        while i < len(bb.instructions):
            inst = bb.instructions[i]
            eng = getattr(inst, "engine", None)
            si = inst.sync_info
            ty = type(inst).__name__
            if eng is None or si is None or len(si.on_wait) < 2:
                i += 1
                continue
            waits = list(si.on_wait)
            p = None  # stripping self-waits proved unsafe on HW; split all
            if p is not None and any(not w.ant_name.startswith(p) for w in waits):
                waits = [w for w in waits if not w.ant_name.startswith(p)]
            while len(waits) > 1:
                bb.instructions.insert(i, make_wait(eng, waits.pop(0)))
                i += 1
            si.on_wait[:] = waits
            i += 1


def make_in_maps(pred, target, ncores=NCORES, F=196):
    """Shard + repack host side. pred/target: [B,S,S,C] f32 np arrays."""
    bs = pred.shape[0] // ncores
    in_maps = []
    for i in range(ncores):
        pf = pred[i * bs:(i + 1) * bs].reshape(-1, C)
        tf = target[i * bs:(i + 1) * bs].reshape(-1, C)
        boxm = np.concatenate([pf[:, :10], tf[:, :10]], axis=1).astype(BF16)
        clsm = np.concatenate([pf[:, 10:], tf[:, 10:]], axis=1).astype(BF16)
        in_maps.append({
            "box": np.ascontiguousarray(boxm).reshape(-1, F, 20),
            "cls": np.ascontiguousarray(clsm).reshape(-1, F, 40),
        })
    return in_maps


def combine(outs, n):
    """outs: list of per-core [P, T*NACC] partial-sum arrays."""
    a = np.stack([o.astype(np.float64) for o in outs])
    s = a.reshape(len(outs), P, -1, NACC).sum(axis=(0, 1, 2))
    nooobj = s[0] - s[1]
    cls_sum = s[2]
    loc = s[3] + s[4]
    contain = s[5] + s[6]
    notcont = s[7] + s[8]
    total = (5.0 * loc + 2.0 * contain + notcont + 0.5 * nooobj + cls_sum) / n
    return (np.float32(total), np.float32(loc), np.float32(contain),
            np.float32(nooobj), np.float32(cls_sum))


_NC_CACHE = {}


def _get_nc():
    if "nc" not in _NC_CACHE:
        _NC_CACHE["nc"] = build_nc()
    return _NC_CACHE["nc"]


def run(in_maps, nc=None, **kw):
    if nc is None:
        nc = _get_nc()
    return run_bass_kernel_spmd(nc, in_maps, core_ids=list(range(len(in_maps))), **kw)


def kernel(pred, target):
    pred = np.asarray(pred, dtype=np.float32)
    target = np.asarray(target, dtype=np.float32)
    in_maps = make_in_maps(pred, target)
    res = run(in_maps)
    return combine([r["out"] for r in res.results], pred.shape[0])


if __name__ == "__main__":
    rng = np.random.default_rng(0)
    pred = rng.uniform(0.01, 1.0, (B, S, S, C)).astype(np.float32)
    target = rng.uniform(0.01, 1.0, (B, S, S, C)).astype(np.float32)
    target[..., 4] = (rng.uniform(size=(B, S, S)) < 0.1).astype(np.float32)
    print(kernel(pred, target))



# revision 3
# speedup vs baseline: 1.0418x; 1.0418x over previous
"""YOLOv1 loss kernel for 8 Trainium2 NeuronCores.

Strategy (data parallel over batch):
  - Shard pred/target [16384,7,7,30] along batch across 8 cores (2048 each).
  - Per core: 2048*49 = 100352 cells, each a 30-float record.
    Split channels: "box" block ch0:10 and "class" block ch10:30 both
    downcast to bf16 on host (cuts DMA bytes from 240 to 120 per cell,
    and bf16 step-1 operands let tensor_tensor ops hit the DVE 2x_1p
    perf mode; worst-case output rel err ~5e-4 vs the 2e-2 gate).
  - Layout: cells mapped to [T=4 tiles][P=128 partitions][F=196 cells].
    All per-cell math is [P, F]-shaped elementwise work spread across
    DVE / ACT / Pool engines; per-(partition,tile) partial sums of the
    9 loss sub-terms are emitted via accum_out into a [P, T*10] tile,
    DMA'd out once. Host does the final tiny reduction in float64.

IoU via interval-overlap identity:
  overlap_x = relu(0.5*(wp+wt) - |cxp - cxt|)  (centers pre-scaled by 1/14)
Responsible-box select: sel = (iou1 > iou0) as 0/1 mask; every
sel-dependent loss term is accumulated separately for box0 (mask
coo*(1-sel)) and box1 (mask coo*sel), so no blend ops are needed.
"""

import sys

if "/opt/trn_rl_repo" not in sys.path:
    sys.path.insert(0, "/opt/trn_rl_repo")

import numpy as np
import ml_dtypes

import concourse.bass as bass
import concourse.tile as tile
from concourse import mybir
from concourse.bass_utils import run_bass_kernel_spmd

BF16 = ml_dtypes.bfloat16

NCORES = 8
B, S, C = 16384, 7, 30
BS = B // NCORES            # 2048 batches per core
NCELL = BS * S * S          # 100352 cells per core
P = 128
FPP = NCELL // P            # 784 cells per partition
INV = 1.0 / 14.0
NACC = 10                   # accum columns per tile (9 used)

f32 = mybir.dt.float32
bf16 = mybir.dt.bfloat16
Alu = mybir.AluOpType
Act = mybir.ActivationFunctionType
Ax = mybir.AxisListType


def build_nc(ncell=NCELL, F=196, fix_waits=True, repeat=0, level=4):
    F2 = 2 * F
    """Build the SPMD Bass program for one core's shard of `ncell` cells."""
    T = ncell // (P * F)
    assert T * P * F == ncell

    nc = bass.Bass(target_bir_lowering=False)
    # merged streams: box = pred ch0:10 | target ch0:10 (f32);
    # cls = pred ch10:30 | target ch10:30 (bf16)
    box = nc.declare_dram_parameter("box", [T * P, F, 20], bf16, isOutput=False)
    clsin = nc.declare_dram_parameter("cls", [T * P, F, 40], bf16, isOutput=False)
    out = nc.declare_dram_parameter("out", [P, T * NACC], f32, isOutput=True)

    ENG_SUB = nc.vector
    ENG_ADD = nc.vector
    ENG_STT = nc.vector
    with tile.TileContext(nc) as tc:
        with tc.tile_pool(name="io", bufs=2) as io, \
             tc.tile_pool(name="tmp", bufs=1) as tmp, \
             tc.tile_pool(name="accp", bufs=1) as accp:
            acc = accp.tile([P, T * NACC], f32)
            nc.vector.memset(acc, 0.0)

            import contextlib
            assert T % 2 == 0
            rep_ctx = tc.For_i(0, repeat, 1) if repeat else contextlib.nullcontext()
            with rep_ctx:
                for it in range(T // 2):
                    r0, r1, r2 = 2 * it * P, (2 * it + 1) * P, (2 * it + 2) * P
                    F = F2
                    mb = io.tile([P, F, 20], bf16, tag="mb")
                    mc = io.tile([P, F, 40], bf16, tag="mc")
                    if level != 5:
                        nc.sync.dma_start(out=mb[:, 0:F // 2, :], in_=box[r0:r1])
                        nc.sync.dma_start(out=mb[:, F // 2:F, :], in_=box[r1:r2])
                        nc.sync.dma_start(out=mc[:, 0:F // 2, :], in_=clsin[r0:r1])
                        nc.sync.dma_start(out=mc[:, F // 2:F, :], in_=clsin[r1:r2])
                    pbt = mb[:, :, 0:10]
                    tbt = mb[:, :, 10:20]
                    scrV = tmp.tile([P, F], bf16, tag="scrV")
                    scrA = scrV
                    scrP = scrV
                    b = it * NACC

                    def vacc(mask, val, col):
                        nc.vector.scalar_tensor_tensor(
                            scrV, mask, 0.0, val, op0=Alu.bypass, op1=Alu.mult,
                            accum_out=acc[:, col:col + 1])

                    coo = tbt[:, :, 4]

                    if level == 0:
                        continue
                    # ---- class block: q = (p-t)^2, squared into a fully
                    # contiguous bf16 tile so the reduce can use packed mode
                    qc = mc[:, :, 20:40]
                    ENG_SUB.tensor_sub(qc, mc[:, :, 0:20], qc)
                    nc.scalar.square(qc, qc)
                    cls = tmp.tile([P, F], f32, tag="cls")
                    nc.vector.tensor_reduce(cls, qc, axis=Ax.X, op=Alu.add)

                    if level >= 1:
                        vacc(mb[:, :, 14], cls, b + 2)
                    if level == 1:
                        continue
                    # ---- box diffs packed: dxy=(d0,d1,d5,d6), dd49=(d4,d9)
                    dxy = tmp.tile([P, F, 4], bf16, tag="dxy")
                    nc.vector.tensor_sub(dxy[:, :, 0:2], pbt[:, :, 0:2], tbt[:, :, 0:2])
                    nc.vector.tensor_sub(dxy[:, :, 2:4], pbt[:, :, 5:7], tbt[:, :, 5:7])
                    dd49 = tmp.tile([P, F, 2], bf16, tag="dd49")
                    nc.vector.tensor_sub(dd49, mb[:, :, 4:10:5], mb[:, :, 14:20:5])
                    e1 = tmp.tile([P, F, 2], bf16, tag="e1")
                    nc.vector.tensor_sub(e1, pbt[:, :, 5:7], tbt[:, :, 0:2])

                    # center distances |d|/14 (read raw diffs before in-place squares)
                    av = tmp.tile([P, F, 4], bf16, tag="av")
                    nc.scalar.activation(av[:, :, 0:2], dxy[:, :, 0:2], Act.Abs, scale=INV)
                    nc.scalar.activation(av[:, :, 2:4], e1, Act.Abs, scale=INV)

                    nc.scalar.square(dxy, dxy)
                    nc.scalar.square(dd49, dd49)

                    f01 = tmp.tile([P, F, 2], bf16, tag="f01")   # xy sq-dist per box
                    nc.vector.tensor_add(f01, dxy[:, :, 0:4:2], dxy[:, :, 1:4:2])
                    snn = tmp.tile([P, F], bf16, tag="snn")
                    nc.vector.tensor_add(snn, dd49[:, :, 0], dd49[:, :, 1])

                    if level == 2:
                        vacc(mb[:, :, 14], snn, b + 1)
                        vacc(mb[:, :, 14], f01[:, :, 0], b + 3)
                        continue

                    # ---- intersection: iw = relu(min(0.5*(wp+wt) - |dc|, wp, wt))
                    sw = tmp.tile([P, F, 4], bf16, tag="sw")     # (b0x,b0y,b1x,b1y)
                    nc.vector.tensor_add(sw[:, :, 0:2], pbt[:, :, 2:4], tbt[:, :, 2:4])
                    nc.vector.tensor_add(sw[:, :, 2:4], pbt[:, :, 7:9], tbt[:, :, 2:4])
                    nc.vector.scalar_tensor_tensor(sw, sw, 0.5, av,
                                                   op0=Alu.mult, op1=Alu.subtract)
                    nc.vector.tensor_tensor(sw[:, :, 0:2], sw[:, :, 0:2], pbt[:, :, 2:4], op=Alu.min)
                    nc.vector.tensor_tensor(sw[:, :, 2:4], sw[:, :, 2:4], pbt[:, :, 7:9], op=Alu.min)
                    nc.vector.tensor_tensor(sw[:, :, 0:2], sw[:, :, 0:2], tbt[:, :, 2:4], op=Alu.min)
                    nc.vector.tensor_tensor(sw[:, :, 2:4], sw[:, :, 2:4], tbt[:, :, 2:4], op=Alu.min)
                    nc.scalar.activation(sw, sw, Act.Relu)
                    inter = tmp.tile([P, F, 2], bf16, tag="inter")
                    nc.vector.tensor_mul(inter, sw[:, :, 0:4:2], sw[:, :, 1:4:2])

                    at = tmp.tile([P, F], bf16, tag="at")
                    nc.vector.tensor_mul(at, tbt[:, :, 2], tbt[:, :, 3])
                    ap01 = tmp.tile([P, F, 2], bf16, tag="ap01")
                    nc.vector.tensor_mul(ap01, mb[:, :, 2:9:5], mb[:, :, 3:9:5])
                    x01 = tmp.tile([P, F, 2], bf16, tag="x01")   # union
                    nc.vector.tensor_sub(x01[:, :, 0], at, inter[:, :, 0])
                    nc.vector.tensor_sub(x01[:, :, 1], at, inter[:, :, 1])
                    nc.vector.tensor_add(x01, ap01, x01)

                    # log-domain iou: avoids the 8-cyc/elem iterative divide on DVE
                    nc.vector.tensor_scalar_max(inter, inter, 1e-30)
                    lnq = tmp.tile([P, F, 2], bf16, tag="lnq")
                    nc.scalar.activation(lnq, inter, Act.Ln)
                    nc.scalar.activation(x01, x01, Act.Ln)
                    nc.vector.tensor_sub(lnq, lnq, x01)         # log-iou per box
                    sel = tmp.tile([P, F], bf16, tag="sel")
                    nc.vector.tensor_tensor(sel, lnq[:, :, 1], lnq[:, :, 0], op=Alu.is_gt)
                    mlog = tmp.tile([P, F], bf16, tag="mlog")
                    nc.vector.tensor_max(mlog, lnq[:, :, 1], lnq[:, :, 0])
                    miou = tmp.tile([P, F], bf16, tag="miou")
                    nc.scalar.activation(miou, mlog, Act.Exp)

                    # contain: (conf_b - max_iou)^2 per box
                    c01 = tmp.tile([P, F, 2], bf16, tag="c01")
                    nc.vector.tensor_sub(c01[:, :, 0], pbt[:, :, 4], miou)
                    nc.vector.tensor_sub(c01[:, :, 1], pbt[:, :, 9], miou)
                    nc.scalar.square(c01, c01)

                    if level == 3:
                        cs1x = tmp.tile([P, F], bf16, tag="cs1x")
                        nc.vector.tensor_mul(cs1x, mb[:, :, 14], sel)
                        vacc(cs1x, c01[:, :, 0], b + 5)
                        vacc(cs1x, snn, b + 1)
                        continue

                    # not-contain: stored cross-paired (p9^2, p4^2) so one stt works
                    npk = tmp.tile([P, F, 2], bf16, tag="npk")
                    nc.scalar.square(npk[:, :, 0], pbt[:, :, 9])
                    nc.scalar.square(npk[:, :, 1], pbt[:, :, 4])

                    # wh: (sqrt(p)-sqrt(t))^2, both boxes packed
                    # sqrt via exp(0.5*ln x): stays in the natural_log_exp
                    # ACT table set (a direct Sqrt would force a ~2.7us
                    # table-set reload per tile)
                    spq = tmp.tile([P, F, 4], bf16, tag="spq")
                    nc.scalar.activation(spq[:, :, 0:2], pbt[:, :, 2:4], Act.Ln)
                    nc.scalar.activation(spq[:, :, 2:4], pbt[:, :, 7:9], Act.Ln)
                    nc.scalar.activation(spq, spq, Act.Exp, scale=0.5)
                    stq = tmp.tile([P, F, 4], bf16, tag="stq")
                    nc.scalar.activation(stq[:, :, 0:2], tbt[:, :, 2:4], Act.Ln)
                    nc.scalar.activation(stq[:, :, 2:4], tbt[:, :, 7:9], Act.Ln)
                    nc.scalar.activation(stq, stq, Act.Exp, scale=0.5)
                    nc.vector.tensor_sub(spq, spq, stq)
                    nc.scalar.square(spq, spq)
                    w01 = tmp.tile([P, F, 2], bf16, tag="w01")
                    nc.vector.tensor_add(w01, spq[:, :, 0:4:2], spq[:, :, 1:4:2])
                    nc.vector.tensor_add(f01, f01, w01)         # loc per box

                    # masks: cs01 = (coo*(1-sel), coo*sel)
                    cs01 = lnq
                    nc.vector.tensor_mul(cs01[:, :, 1], coo, sel)
                    nc.vector.tensor_sub(cs01[:, :, 0], coo, cs01[:, :, 1])

                    # ---- partial sums (accum_out sums the whole free dim, so one
                    # [P,F,2] stt folds both boxes' masked terms into one column)
                    scr2 = tmp.tile([P, F, 2], bf16, tag="scr2")
                    nc.scalar.activation(scrA, snn, Act.Copy, accum_out=acc[:, b + 0:b + 1])
                    vacc(coo, snn, b + 1)
                    vacc(coo, cls, b + 2)
                    nc.vector.scalar_tensor_tensor(
                        scr2, cs01, 0.0, f01, op0=Alu.bypass, op1=Alu.mult,
                        accum_out=acc[:, b + 3:b + 4])
                    nc.vector.scalar_tensor_tensor(
                        scr2, cs01, 0.0, c01, op0=Alu.bypass, op1=Alu.mult,
                        accum_out=acc[:, b + 5:b + 6])
                    nc.vector.scalar_tensor_tensor(
                        scr2, cs01, 0.0, npk, op0=Alu.bypass, op1=Alu.mult,
                        accum_out=acc[:, b + 7:b + 8])

            nc.gpsimd.dma_start(out=out[:, :], in_=acc[:, :])
    if fix_waits:
        _fix_multi_waits(nc)
    return nc


def _fix_multi_waits(nc):
    """Work around a walrus codegen limit: one sync-wait per TPB instruction.

    Tile sometimes attaches 2-3 sem waits to one engine instruction, which
    this toolchain rejects ("Too many sync wait commands"). Two-step fix:
    1. Drop own-engine waits on DVE/Pool ops (those engines execute their
       streams serially -- DVE drains after every op -- so program order
       already guarantees them).
    2. For any remaining multi-wait compute instruction, move all but the
       last wait onto injected same-engine nops placed immediately before
       it (the engine's sequencer executes them in order, so semantics are
       identical).
    """
    import concourse.mybir as _mybir
    from bass_rust import SyncInfo

    self_pfx = {_mybir.EngineType.DVE: "DVE_", _mybir.EngineType.Pool: "Pool_"}
    blocks = [bb for fn in nc.m.functions for bb in fn.blocks]
    nseq = [0]

    def make_wait(eng, w):
        nseq[0] += 1
        ev = _mybir.InstEventSemaphore(name=f"W-split-{nseq[0]}")
        ev.engine = eng
        ev.sync_info = SyncInfo(on_wait=[w], on_update=[])
        return ev

    for bb in blocks:
        i = 0
        while i < len(bb.instructions):
            inst = bb.instructions[i]
            eng = getattr(inst, "engine", None)
            si = inst.sync_info
            ty = type(inst).__name__
            if eng is None or si is None or len(si.on_wait) < 2:
                i += 1
                continue
            waits = list(si.on_wait)
            p = None  # stripping self-waits proved unsafe on HW; split all
            if p is not None and any(not w.ant_name.startswith(p) for w in waits):
                waits = [w for w in waits if not w.ant_name.startswith(p)]
            while len(waits) > 1:
                bb.instructions.insert(i, make_wait(eng, waits.pop(0)))
                i += 1
            si.on_wait[:] = waits
            i += 1


def make_in_maps(pred, target, ncores=NCORES, F=196):
    """Shard + repack host side. pred/target: [B,S,S,C] f32 np arrays."""
    bs = pred.shape[0] // ncores
    in_maps = []
    for i in range(ncores):
        pf = pred[i * bs:(i + 1) * bs].reshape(-1, C)
        tf = target[i * bs:(i + 1) * bs].reshape(-1, C)
        boxm = np.concatenate([pf[:, :10], tf[:, :10]], axis=1).astype(BF16)
        clsm = np.concatenate([pf[:, 10:], tf[:, 10:]], axis=1).astype(BF16)
        in_maps.append({
            "box": np.ascontiguousarray(boxm).reshape(-1, F, 20),
            "cls": np.ascontiguousarray(clsm).reshape(-1, F, 40),
        })
    return in_maps


def combine(outs, n):
    """outs: list of per-core [P, T*NACC] partial-sum arrays."""
    a = np.stack([o.astype(np.float64) for o in outs])
    s = a.reshape(len(outs), P, -1, NACC).sum(axis=(0, 1, 2))
    nooobj = s[0] - s[1]
    cls_sum = s[2]
    loc = s[3] + s[4]
    contain = s[5] + s[6]
    notcont = s[7] + s[8]
    total = (5.0 * loc + 2.0 * contain + notcont + 0.5 * nooobj + cls_sum) / n
    return (np.float32(total), np.float32(loc), np.float32(contain),
            np.float32(nooobj), np.float32(cls_sum))


_NC_CACHE = {}


def _get_nc():
    if "nc" not in _NC_CACHE:
        _NC_CACHE["nc"] = build_nc()
    return _NC_CACHE["nc"]


def run(in_maps, nc=None, **kw):
    if nc is None:
        nc = _get_nc()
    return run_bass_kernel_spmd(nc, in_maps, core_ids=list(range(len(in_maps))), **kw)


def kernel(pred, target):
    pred = np.asarray(pred, dtype=np.float32)
    target = np.asarray(target, dtype=np.float32)
    in_maps = make_in_maps(pred, target)
    res = run(in_maps)
    return combine([r["out"] for r in res.results], pred.shape[0])


if __name__ == "__main__":
    rng = np.random.default_rng(0)
    pred = rng.uniform(0.01, 1.0, (B, S, S, C)).astype(np.float32)
    target = rng.uniform(0.01, 1.0, (B, S, S, C)).astype(np.float32)
    target[..., 4] = (rng.uniform(size=(B, S, S)) < 0.1).astype(np.float32)
    print(kernel(pred, target))

